# revision 13
# baseline (speedup 1.0000x reference)
"""3-layer GCN (gcn_norm message passing) on 8 Trainium2 NeuronCores.

Strategy (graph/data parallel, edges partitioned by destination node):
  - Nodes are row-sharded across the 8 cores (12500 real + 44 pad rows each).
  - Per layer, each core computes h_mm = relu(h_prev) @ W for its own shard,
    AllGathers it so every core holds the full [100352, 128] table in DRAM
    (bf16), then aggregates messages for the destinations it owns.
  - Edges are pre-sorted by (dest-block, source-quarter) on the host. Source
    rows are fetched with dma_gather (custom Q7 SWDGE gather, int16 indices,
    so the table is addressed through 4 row-windows of <=25088 rows), one
    call per (4-block group, quarter).
  - The segment-sum runs on the TensorEngine: for each 128-message chunk,
    psum[feat, dest] += msgs[msg, feat].T-contraction with a norm-scaled
    one-hot [msg, dest] built in one fused DVE op ((iota == seg) * norm).
  - Bias/relu/next-layer matmul hang directly off the accumulated PSUM tile.

All data-dependent structure (per-(block, quarter) chunk capacities, max'd
across cores so the SPMD program is shared) is baked at trace time inside
kernel(); the NEFF is compiled per call and cached in-process.
"""

import os
import sys

sys.path.insert(0, "/opt/trn_rl_repo")

import numpy as np

from concourse import bacc, bass, mybir
from concourse import tile
from concourse import bass_utils

F32 = mybir.dt.float32
BF16 = mybir.dt.bfloat16
I16 = mybir.dt.int16

N_CORES = 8
D = 128
NQ = 4  # source windows (int16 index range / table rows)


def _layout(caps, G):
    """Shared prep/builder enumeration.

    caps: [nblk][NQ] chunk capacities (uniform across cores).
    Returns dict with per-call and per-(b,q) offsets. Call order is
    (group asc, quarter asc); chunk order is (group, quarter, block, chunk).
    """
    nblk = len(caps)
    ngrp = (nblk + G - 1) // G
    call_cols = {}       # (g, q) -> gidx col base
    call_nidx = {}       # (g, q) -> num_idxs
    chunk_base = {}      # (b, q) -> global chunk index base
    run_chunk = {}       # (b, q) -> chunk offset of block's run inside its call
    col = 0
    chk = 0
    for g in range(ngrp):
        blocks = range(g * G, min((g + 1) * G, nblk))
        for q in range(NQ):
            call_cols[(g, q)] = col
            off = 0
            for b in blocks:
                run_chunk[(b, q)] = off
                chunk_base[(b, q)] = chk + off
                off += caps[b][q]
            call_nidx[(g, q)] = off * 128
            col += off * 8  # int16 cols = num_idxs / 16
            chk += off
    return {
        "ngrp": ngrp,
        "call_cols": call_cols,
        "call_nidx": call_nidx,
        "chunk_base": chunk_base,
        "run_chunk": run_chunk,
        "gidx_cols": col,
        "total_chunks": chk,
    }


# ----------------------------------------------------------------------------
# Host-side preparation
# ----------------------------------------------------------------------------

def _prep_inputs(x, edge_index, W0, b0, W1, b1, W2, b2, s_real, G=4):
    n = x.shape[0]
    assert n % N_CORES == 0 and s_real == n // N_CORES
    nblk = (s_real + 127) // 128
    s_pad = nblk * 128
    total = N_CORES * s_pad
    assert total % NQ == 0
    wq = total // NQ
    assert wq <= 32767, f"window {wq} exceeds int16 range"

    d = np.asarray(edge_index[0], dtype=np.int64)
    s = np.asarray(edge_index[1], dtype=np.int64)

    deg = np.bincount(s, minlength=n).astype(np.float64) + 1.0
    dis = 1.0 / np.sqrt(deg)

    dests = np.concatenate([d, np.arange(n, dtype=np.int64)])
    srcs = np.concatenate([s, np.arange(n, dtype=np.int64)])
    w = (dis[dests] * dis[srcs]).astype(np.float32)

    core = dests // s_real
    local = dests - core * s_real
    blk = local >> 7
    seg = (local & 127).astype(np.float32)
    sg = (srcs // s_real) * s_pad + (srcs % s_real)  # padded-global row
    q = sg // wq
    widx = (sg - q * wq).astype(np.int64)

    # counts per (core, block, quarter)
    key_cbq = (core * nblk + blk) * NQ + q
    counts = np.bincount(key_cbq, minlength=N_CORES * nblk * NQ).reshape(
        N_CORES, nblk, NQ
    )
    caps_arr = (counts.max(axis=0) + 127) // 128  # [nblk, NQ]
    caps_arr = np.maximum(caps_arr, 1)
    caps = caps_arr.tolist()

    lay = _layout(caps, G)

    # rank of each message within its (core, block, quarter)
    order = np.argsort(key_cbq, kind="stable")
    inv = np.empty_like(order)
    inv[order] = np.arange(order.size)
    starts = np.zeros(N_CORES * nblk * NQ + 1, dtype=np.int64)
    np.cumsum(np.bincount(key_cbq, minlength=N_CORES * nblk * NQ), out=starts[1:])
    rank = inv - starts[key_cbq]

    # destination slot of each message
    cb = np.array(
        [[lay["chunk_base"][(b, qq)] for qq in range(NQ)] for b in range(nblk)],
        dtype=np.int64,
    )
    rc = np.array(
        [[lay["run_chunk"][(b, qq)] for qq in range(NQ)] for b in range(nblk)],
        dtype=np.int64,
    )
    ccols = np.zeros((lay["ngrp"], NQ), dtype=np.int64)
    for (g, qq), c in lay["call_cols"].items():
        ccols[g, qq] = c

    part = (rank & 127).astype(np.int64)
    chunk_in_run = rank >> 7
    # gidx position: call col base + (run chunk base*128 + rank) wrapped %16
    slot_in_call = rc[blk, q] * 128 + rank
    gcol = ccols[blk // G, q] + (slot_in_call >> 4)
    grow = slot_in_call & 15
    # meta position: global chunk id
    gchunk = cb[blk, q] + chunk_in_run

    gidx16 = np.zeros((N_CORES, 16, lay["gidx_cols"]), dtype=np.int16)
    meta = np.zeros((N_CORES, 128, 2 * lay["total_chunks"]), dtype=np.float32)

    gidx16[core, grow, gcol] = widx.astype(np.int16)
    meta[core, part, 2 * gchunk] = seg
    meta[core, part, 2 * gchunk + 1] = w

    gidx = np.broadcast_to(
        gidx16[:, None, :, :], (N_CORES, 8, 16, lay["gidx_cols"])
    ).reshape(N_CORES, 128, lay["gidx_cols"]).copy()

    x = np.asarray(x, dtype=np.float32)
    x_t = np.zeros((N_CORES, 128, s_pad), dtype=np.float32)
    for r in range(N_CORES):
        x_t[r, :, :s_real] = x[r * s_real : (r + 1) * s_real].T

    wdata = np.zeros((128, 3 * 128 + 3 + 128), dtype=np.float32)
    wdata[:, 0:128] = np.asarray(W0, dtype=np.float32)
    wdata[:, 128:256] = np.asarray(W1, dtype=np.float32)
    wdata[:, 256:384] = np.asarray(W2, dtype=np.float32)
    wdata[:, 384] = np.asarray(b0, dtype=np.float32)
    wdata[:, 385] = np.asarray(b1, dtype=np.float32)
    wdata[:, 386] = np.asarray(b2, dtype=np.float32)
    wdata[:, 387:515] = np.arange(128, dtype=np.float32)[None, :]

    in_maps = [
        {"x_t": x_t[r], "meta": meta[r], "gidx": gidx[r], "wdata": wdata}
        for r in range(N_CORES)
    ]
    mi = {"nblk": nblk, "s_pad": s_pad, "s_real": s_real, "G": G}
    return in_maps, caps, mi


# ----------------------------------------------------------------------------
# Device kernel builder
# ----------------------------------------------------------------------------

def build_kernel(caps, nblk, s_pad, G=4, n_cores=N_CORES):
    from contextlib import ExitStack

    lay = _layout(caps, G)
    total = n_cores * s_pad
    wq = total // NQ

    nc = bacc.Bacc(
        "TRN2", target_bir_lowering=False, debug=False, num_devices=n_cores
    )
    x_t = nc.dram_tensor("x_t", [128, s_pad], F32, kind="ExternalInput")
    meta = nc.dram_tensor(
        "meta", [128, 2 * lay["total_chunks"]], F32, kind="ExternalInput"
    )
    gidx = nc.dram_tensor("gidx", [128, lay["gidx_cols"]], I16, kind="ExternalInput")
    wdata = nc.dram_tensor("wdata", [128, 3 * 128 + 3 + 128], F32, kind="ExternalInput")
    h_out = nc.dram_tensor("h_out", [128, 3 * s_pad], F32, kind="ExternalOutput")

    rg = [list(range(n_cores))]

    with tile.TileContext(nc) as tc, ExitStack() as ctx:
        const = ctx.enter_context(tc.tile_pool(name="const", bufs=1))
        dram = ctx.enter_context(tc.tile_pool(name="dram", bufs=1, space="DRAM"))
        xw = ctx.enter_context(tc.tile_pool(name="xw", bufs=4))
        hmm = ctx.enter_context(tc.tile_pool(name="hmm", bufs=4))
        gath = ctx.enter_context(tc.tile_pool(name="gath", bufs=2 * NQ))
        idxp = ctx.enter_context(tc.tile_pool(name="idxp", bufs=2 * NQ))
        metat = ctx.enter_context(tc.tile_pool(name="metat", bufs=3))
        ohp = ctx.enter_context(tc.tile_pool(name="ohp", bufs=6))
        outsb = ctx.enter_context(tc.tile_pool(name="outsb", bufs=3))
        rsb = ctx.enter_context(tc.tile_pool(name="rsb", bufs=3))
        agg_ps = ctx.enter_context(tc.tile_pool(name="agg_ps", bufs=3, space="PSUM"))
        mm_ps = ctx.enter_context(tc.tile_pool(name="mm_ps", bufs=2, space="PSUM"))

        ag_in = dram.tile([s_pad, 128], BF16)
        ag_outs = [
            dram.tile([total, 128], BF16, addr_space="Shared", name=f"ag_out_l{i}")
            for i in range(3)
        ]

        w_sb = const.tile([128, 3 * 128 + 3 + 128], F32)
        nc.sync.dma_start(out=w_sb[:], in_=wdata[:])
        w_bf = const.tile([128, 3 * 128], BF16)
        nc.vector.tensor_copy(w_bf[:], w_sb[:, 0 : 3 * 128])
        iota_bf = w_sb[:, 387:515]

        def bias(L):
            return w_sb[:, 384 + L : 385 + L]

        # ---- Phase A: h0_mm = x @ W0 for own shard -> ag_in (bf16) ----
        for b in range(nblk):
            xt = xw.tile([128, 128], F32)
            nc.sync.dma_start(out=xt[:], in_=x_t[:, b * 128 : (b + 1) * 128])
            ps = mm_ps.tile([128, 128], F32)
            nc.tensor.matmul(
                ps[:], lhsT=xt[:], rhs=w_sb[:, 0:128], start=True, stop=True
            )
            hm = hmm.tile([128, 128], BF16)
            nc.vector.tensor_copy(hm[:], ps[:])
            nc.sync.dma_start(out=ag_in[b * 128 : (b + 1) * 128, :], in_=hm[:])

        # ---- 3 layers ----
        no_cc = os.environ.get("TRN_NO_CC", "") == "1"
        for L in range(3):
            ag_out = ag_outs[L]
            if no_cc:
                nc.sync.dma_start(out=ag_out[0:s_pad, :], in_=ag_in[:])
            else:
                nc.gpsimd.collective_compute(
                    "AllGather",
                    mybir.AluOpType.bypass,
                    replica_groups=rg,
                    ins=[ag_in[:].opt()],
                    outs=[ag_out[:].opt()],
                )
            for g in range(lay["ngrp"]):
                blocks = list(range(g * G, min((g + 1) * G, nblk)))
                # 4 windowed gathers for this group
                gts = []
                for q in range(NQ):
                    nidx = lay["call_nidx"][(g, q)]
                    c0 = lay["call_cols"][(g, q)]
                    it = idxp.tile([128, nidx // 16], I16, name=f"it{q}", tag="it")
                    nc.sync.dma_start(
                        out=it[:], in_=gidx[:, c0 : c0 + nidx // 16]
                    )
                    gt = gath.tile([128, (nidx // 128) * 128], BF16, name=f"gt{q}", tag="gt")
                    nc.gpsimd.dma_gather(
                        gt[:].rearrange("p (c f) -> p c f", f=128),
                        ag_out[q * wq : (q + 1) * wq, :],
                        it[:],
                        num_idxs=nidx,
                        num_idxs_reg=nidx,
                        elem_size=128,
                        elem_step=128,
                        single_packet=(nidx <= 1024),
                    )
                    gts.append(gt)
                # chunk meta for the whole group: [seg, norm] per chunk
                ch0 = lay["chunk_base"][(blocks[0], 0)]
                gchunks = sum(caps[b][q] for b in blocks for q in range(NQ))
                mt = metat.tile([128, 2 * gchunks], F32)
                nc.sync.dma_start(
                    out=mt[:], in_=meta[:, 2 * ch0 : 2 * (ch0 + gchunks)]
                )
                strip = os.environ.get("TRN_STRIP", "full")
                if strip == "gather":
                    continue
                # aggregate each block of the group
                for b in blocks:
                    ps = agg_ps.tile([128, 128], F32, name="aggps")
                    n_mm = sum(caps[b][q] for q in range(NQ))
                    k = 0
                    for q in range(NQ):
                        rbase = lay["run_chunk"][(b, q)]
                        mbase = lay["chunk_base"][(b, q)] - ch0
                        for c in range(caps[b][q]):
                            oh = ohp.tile([128, 128], BF16, name="oh")
                            mcol = 2 * (mbase + c)
                            nc.vector.tensor_scalar(
                                oh[:],
                                iota_bf,
                                mt[:, mcol : mcol + 1],
                                mt[:, mcol + 1 : mcol + 2],
                                mybir.AluOpType.is_equal,
                                mybir.AluOpType.mult,
                            )
                            cc = rbase + c
                            nc.tensor.matmul(
                                ps[:],
                                lhsT=gts[q][:, cc * 128 : (cc + 1) * 128],
                                rhs=oh[:],
                                start=(k == 0),
                                stop=(k == n_mm - 1),
                            )
                            k += 1
                    if strip == "mm_only":
                        continue
                    # psum is [feat, dest]; bias is per-partition (= feature)
                    ob = outsb.tile([128, 128], F32)
                    nc.vector.tensor_scalar_add(ob[:], ps[:], bias(L))
                    nc.sync.dma_start(
                        out=h_out[
                            :, L * s_pad + b * 128 : L * s_pad + (b + 1) * 128
                        ],
                        in_=ob[:],
                    )
                    if L < 2 and strip != "no_epi":
                        r = rsb.tile([128, 128], BF16)
                        nc.scalar.activation(
                            r[:], ps[:], mybir.ActivationFunctionType.Relu,
                            bias=bias(L),
                        )
                        ps2 = mm_ps.tile([128, 128], F32)
                        nc.tensor.matmul(
                            ps2[:],
                            lhsT=r[:],
                            rhs=w_bf[:, (L + 1) * 128 : (L + 2) * 128],
                            start=True,
                            stop=True,
                        )
                        hm = hmm.tile([128, 128], BF16)
                        nc.vector.tensor_copy(hm[:], ps2[:])
                        nc.sync.dma_start(
                            out=ag_in[b * 128 : (b + 1) * 128, :], in_=hm[:]
                        )

    nc.compile()
    return nc


_BUILD_CACHE = {}


def _get_kernel(caps, nblk, s_pad, G):
    key = (tuple(tuple(c) for c in caps), nblk, s_pad, G)
    if key not in _BUILD_CACHE:
        _BUILD_CACHE[key] = build_kernel(caps, nblk, s_pad, G=G)
    return _BUILD_CACHE[key]


# ----------------------------------------------------------------------------
# Entry point
# ----------------------------------------------------------------------------

def _run(x, edge_index, W0, b0, W1, b1, W2, b2, trace=False, G=4):
    n = int(np.asarray(x).shape[0])
    s_real = n // N_CORES
    in_maps, caps, mi = _prep_inputs(
        x, edge_index, W0, b0, W1, b1, W2, b2, s_real, G=G
    )
    nblk, s_pad = mi["nblk"], mi["s_pad"]
    nc = _get_kernel(caps, nblk, s_pad, G)
    res = bass_utils.run_bass_kernel_spmd(
        nc, in_maps, core_ids=list(range(N_CORES)), trace=trace
    )
    outs = []
    for L in range(3):
        h = np.concatenate(
            [
                res.results[r]["h_out"][:, L * s_pad : L * s_pad + s_real]
                for r in range(N_CORES)
            ],
            axis=1,
        ).T
        outs.append(h)
    full = np.stack(outs, axis=1).astype(np.float32)
    return full, res


def kernel(**inputs):
    trace = os.environ.get("TRN_KERNEL_TRACE", "") == "1"
    out, res = _run(
        np.asarray(inputs["x"]),
        np.asarray(inputs["edge_index"]),
        np.asarray(inputs["W0"]),
        np.asarray(inputs["b0"]),
        np.asarray(inputs["W1"]),
        np.asarray(inputs["b1"]),
        np.asarray(inputs["W2"]),
        np.asarray(inputs["b2"]),
        trace=trace,
    )
    if trace and res.exec_time_ns is not None:
        print(f"HW exec time: {res.exec_time_ns} ns")
        if res.instructions_and_trace:
            print(f"trace: {res.instructions_and_trace[1]}")
    return out


# revision 14
# speedup vs baseline: 1.6062x; 1.6062x over previous
"""3-layer GCN (gcn_norm message passing) on 8 Trainium2 NeuronCores.

Architecture (v3):
  - Nodes row-sharded across 8 cores (12500 real + 44 pad rows each); per
    layer each core computes h_mm = relu(h_prev) @ W for its shard, scaled by
    dis[src] (norm factorization: norm = dis[dest]*dis[src]), AllGathers the
    bf16 table, then aggregates messages for the destinations it owns.
  - Messages sorted by (dest-group of 4 blocks, source-quarter, dest). One
    dma_gather per (group, quarter) on 4 parallel SWDGE queues (int16 indices
    address the table through 4 row-windows).
  - Segment-sum on the TensorEngine: for each 128-message chunk and each
    destination block it straddles, psum[feat, dest] += msgs^T-contract with
    a 0/1 one-hot. One-hots are built 8 chunks at a time in a single wide
    DVE tensor_tensor (iota_rep == seg broadcast), ~146 ns/chunk.
  - Per-block epilogue: out = psum * dis[dest] + bias (DVE), relu (ACT),
    next-layer matmul (PE), hm = psum2 * dis[own] -> ag_in (DVE).

All data-dependent structure (per-(group, quarter) chunk capacities and the
chunk->block matmul schedule, identical across cores by construction) is
baked at trace time; the NEFF is compiled per call and cached in-process.
"""

import os
import sys

sys.path.insert(0, "/opt/trn_rl_repo")

import numpy as np

from concourse import bacc, bass, mybir
from concourse import tile
from concourse import bass_utils

F32 = mybir.dt.float32
BF16 = mybir.dt.bfloat16
I16 = mybir.dt.int16

N_CORES = 8
NQ = 4       # source windows (int16 index range / table rows)
G = 4        # dest blocks per gather group
WOH = 8      # one-hot chunks per wide DVE op
PAD_SEG = 10000.0


# ----------------------------------------------------------------------------
# Host-side preparation
# ----------------------------------------------------------------------------

def _prep_inputs(x, edge_index, W0, b0, W1, b1, W2, b2, s_real):
    n = x.shape[0]
    assert n % N_CORES == 0 and s_real == n // N_CORES
    nblk = (s_real + 127) // 128
    s_pad = nblk * 128
    total = N_CORES * s_pad
    ngrp = (nblk + G - 1) // G
    assert total % NQ == 0
    wq = total // NQ
    assert wq <= 32767, f"window {wq} exceeds int16 range"

    d = np.asarray(edge_index[0], dtype=np.int64)
    s = np.asarray(edge_index[1], dtype=np.int64)

    deg = np.bincount(s, minlength=n).astype(np.float64) + 1.0
    dis = (1.0 / np.sqrt(deg)).astype(np.float32)

    dests = np.concatenate([d, np.arange(n, dtype=np.int64)])
    srcs = np.concatenate([s, np.arange(n, dtype=np.int64)])

    core = dests // s_real
    dloc = dests - core * s_real           # dest local to core [0, s_real)
    blk = dloc >> 7
    grp = blk // G
    sg = (srcs // s_real) * s_pad + (srcs % s_real)  # padded-global src row
    q = sg // wq
    widx = (sg - q * wq).astype(np.int64)

    # ---- (core, group, quarter) counts -> shared chunk capacities ----
    key = (core * ngrp + grp) * NQ + q
    counts = np.bincount(key, minlength=N_CORES * ngrp * NQ).reshape(
        N_CORES, ngrp, NQ
    )
    caps = np.maximum((counts.max(axis=0) + 127) // 128, 1)  # [ngrp, NQ] chunks

    # ---- slot assignment: sort by (core, grp, q, dest) ----
    sort_key = key * np.int64(s_pad + 1) + dloc
    order = np.argsort(sort_key, kind="stable")
    inv = np.empty_like(order)
    inv[order] = np.arange(order.size)
    starts = np.zeros(N_CORES * ngrp * NQ + 1, dtype=np.int64)
    np.cumsum(counts.reshape(-1), out=starts[1:])
    rank = inv - starts[key]

    call_nidx = caps * 128
    call_cols_flat = np.zeros(ngrp * NQ + 1, dtype=np.int64)
    np.cumsum((call_nidx // 16).reshape(-1), out=call_cols_flat[1:])
    gidx_cols = int(call_cols_flat[-1])

    # gidx: wrapped %16, replicated across 8 core-slices
    ccol = call_cols_flat[grp * NQ + q]
    gcol = ccol + (rank >> 4)
    grow = rank & 15
    gidx16 = np.zeros((N_CORES, 16, gidx_cols), dtype=np.int16)
    gidx16[core, grow, gcol] = widx.astype(np.int16)
    gidx = np.broadcast_to(
        gidx16[:, None, :, :], (N_CORES, 8, 16, gidx_cols)
    ).reshape(N_CORES, 128, gidx_cols).copy()

    # ---- chunk -> dest-block matmul schedule (union across cores) ----
    chunk_in_call = rank >> 7
    part = rank & 127
    chunk_base_flat = np.zeros(ngrp * NQ + 1, dtype=np.int64)
    np.cumsum(caps.reshape(-1), out=chunk_base_flat[1:])
    total_chunks = int(chunk_base_flat[-1])
    gchunk = chunk_base_flat[grp * NQ + q] + chunk_in_call

    jj = blk - grp * G  # block index within group [0, G)
    touched = np.zeros((total_chunks, G), dtype=bool)
    touched[gchunk, jj] = True

    # matmul schedule: per group, j-major then (q, chunk)
    mm_of_group = [[] for _ in range(ngrp)]
    for g in range(ngrp):
        for j in range(G):
            for qq in range(NQ):
                cb = int(chunk_base_flat[g * NQ + qq])
                for c in range(int(caps[g, qq])):
                    if touched[cb + c, j]:
                        mm_of_group[g].append((j, qq, c, cb + c))
    mm_base = np.zeros(ngrp + 1, dtype=np.int64)
    np.cumsum([len(m) for m in mm_of_group], out=mm_base[1:])
    n_mm = int(mm_base[-1])

    mm_col = np.full((total_chunks, G), -1, dtype=np.int64)
    for g in range(ngrp):
        for k, (j, qq, c, gc) in enumerate(mm_of_group[g]):
            mm_col[gc, j] = mm_base[g] + k

    # meta: [128, n_mm] f32; for each message, its chunk's matmul for its own
    # block gets value = dest - block base; everything else stays PAD_SEG
    meta = np.full((N_CORES, 128, n_mm), PAD_SEG, dtype=np.float32)
    col = mm_col[gchunk, jj]
    assert (col >= 0).all()
    meta[core, part, col] = (dloc - blk * 128).astype(np.float32)

    # ---- dense inputs ----
    x = np.asarray(x, dtype=np.float32)
    x_t = np.zeros((N_CORES, 128, s_pad), dtype=np.float32)
    dison = np.zeros((N_CORES, 128, nblk), dtype=np.float32)
    disd = np.zeros((N_CORES, 128, s_pad), dtype=np.float32)
    for r in range(N_CORES):
        x_t[r, :, :s_real] = x[r * s_real : (r + 1) * s_real].T
        dv = np.zeros(s_pad, dtype=np.float32)
        dv[:s_real] = dis[r * s_real : (r + 1) * s_real]
        dison[r] = dv.reshape(nblk, 128).T
        disd[r] = dv[None, :]

    wdata = np.zeros((128, 3 * 128 + 3), dtype=np.float32)
    wdata[:, 0:128] = np.asarray(W0, dtype=np.float32)
    wdata[:, 128:256] = np.asarray(W1, dtype=np.float32)
    wdata[:, 256:384] = np.asarray(W2, dtype=np.float32)
    wdata[:, 384] = np.asarray(b0, dtype=np.float32)
    wdata[:, 385] = np.asarray(b1, dtype=np.float32)
    wdata[:, 386] = np.asarray(b2, dtype=np.float32)
    iotar = np.tile(
        np.arange(128, dtype=np.float32), WOH
    )[None, :].repeat(128, axis=0)

    in_maps = [
        {
            "x_t": x_t[r], "meta": meta[r], "gidx": gidx[r],
            "wdata": wdata, "iotar": iotar, "dison": dison[r],
            "disd": disd[r],
        }
        for r in range(N_CORES)
    ]
    sched = {
        "nblk": nblk, "s_pad": s_pad, "s_real": s_real, "ngrp": ngrp,
        "caps": caps.tolist(),
        "call_cols": call_cols_flat.tolist(),
        "mm_of_group": mm_of_group,
        "mm_base": mm_base.tolist(),
        "n_mm": n_mm,
        "gidx_cols": gidx_cols,
    }
    return in_maps, sched


# ----------------------------------------------------------------------------
# Device kernel builder
# ----------------------------------------------------------------------------

def build_kernel(sched, n_cores=N_CORES):
    from contextlib import ExitStack

    nblk, s_pad, ngrp = sched["nblk"], sched["s_pad"], sched["ngrp"]
    caps = sched["caps"]
    total = n_cores * s_pad
    wq = total // NQ

    nc = bacc.Bacc(
        "TRN2", target_bir_lowering=False, debug=False, num_devices=n_cores,
        num_swdge_queues=NQ,
    )
    x_t = nc.dram_tensor("x_t", [128, s_pad], F32, kind="ExternalInput")
    meta = nc.dram_tensor("meta", [128, sched["n_mm"]], F32, kind="ExternalInput")
    gidx = nc.dram_tensor("gidx", [128, sched["gidx_cols"]], I16, kind="ExternalInput")
    wdata = nc.dram_tensor("wdata", [128, 3 * 128 + 3], F32, kind="ExternalInput")
    iotar = nc.dram_tensor("iotar", [128, WOH * 128], F32, kind="ExternalInput")
    dison = nc.dram_tensor("dison", [128, nblk], F32, kind="ExternalInput")
    disd = nc.dram_tensor("disd", [128, s_pad], F32, kind="ExternalInput")
    h_out = nc.dram_tensor("h_out", [128, 3 * s_pad], F32, kind="ExternalOutput")

    rg = [list(range(n_cores))]

    with tile.TileContext(nc) as tc, ExitStack() as ctx:
        const = ctx.enter_context(tc.tile_pool(name="const", bufs=1))
        dram = ctx.enter_context(tc.tile_pool(name="dram", bufs=1, space="DRAM"))
        xw = ctx.enter_context(tc.tile_pool(name="xw", bufs=4))
        hmm = ctx.enter_context(tc.tile_pool(name="hmm", bufs=4))
        gath = ctx.enter_context(tc.tile_pool(name="gath", bufs=2 * NQ))
        idxp = ctx.enter_context(tc.tile_pool(name="idxp", bufs=2 * NQ))
        metat = ctx.enter_context(tc.tile_pool(name="metat", bufs=3))
        ohp = ctx.enter_context(tc.tile_pool(name="ohp", bufs=6))
        outsb = ctx.enter_context(tc.tile_pool(name="outsb", bufs=3))
        ddp = ctx.enter_context(tc.tile_pool(name="ddp", bufs=3))
        rsb = ctx.enter_context(tc.tile_pool(name="rsb", bufs=3))
        agg_ps = ctx.enter_context(tc.tile_pool(name="agg_ps", bufs=G, space="PSUM"))
        mm_ps = ctx.enter_context(tc.tile_pool(name="mm_ps", bufs=2, space="PSUM"))

        ag_in = dram.tile([s_pad, 128], BF16)
        ag_outs = [
            dram.tile([total, 128], BF16, addr_space="Shared", name=f"ag_out_l{i}")
            for i in range(3)
        ]

        w_sb = const.tile([128, 3 * 128 + 3], F32)
        nc.sync.dma_start(out=w_sb[:], in_=wdata[:])
        w_bf = const.tile([128, 3 * 128], BF16)
        nc.vector.tensor_copy(w_bf[:], w_sb[:, 0 : 3 * 128])
        iota_sb = const.tile([128, WOH * 128], F32)
        nc.sync.dma_start(out=iota_sb[:], in_=iotar[:])
        dison_sb = const.tile([128, nblk], F32)
        nc.sync.dma_start(out=dison_sb[:], in_=dison[:])

        def bias(L):
            return w_sb[:, 384 + L : 385 + L]

        # ---- Phase A: table0 = (x @ W0) * dis -> ag_in ----
        for b in range(nblk):
            xt = xw.tile([128, 128], F32)
            nc.sync.dma_start(out=xt[:], in_=x_t[:, b * 128 : (b + 1) * 128])
            ps = mm_ps.tile([128, 128], F32)
            nc.tensor.matmul(
                ps[:], lhsT=xt[:], rhs=w_sb[:, 0:128], start=True, stop=True
            )
            hm = hmm.tile([128, 128], BF16)
            nc.vector.tensor_scalar(
                hm[:], ps[:], dison_sb[:, b : b + 1], None, mybir.AluOpType.mult
            )
            nc.sync.dma_start(out=ag_in[b * 128 : (b + 1) * 128, :], in_=hm[:])

        # ---- 3 layers ----
        for L in range(3):
            ag_out = ag_outs[L]
            nc.gpsimd.collective_compute(
                "AllGather",
                mybir.AluOpType.bypass,
                replica_groups=rg,
                ins=[ag_in[:].opt()],
                outs=[ag_out[:].opt()],
            )
            for g in range(ngrp):
                blocks = list(range(g * G, min((g + 1) * G, nblk)))
                gts = []
                for q in range(NQ):
                    nidx = caps[g][q] * 128
                    c0 = sched["call_cols"][g * NQ + q]
                    it = idxp.tile([128, nidx // 16], I16, name="it", tag="it")
                    nc.sync.dma_start(
                        out=it[:], in_=gidx[:, c0 : c0 + nidx // 16]
                    )
                    gt = gath.tile([128, nidx], BF16, name="gt", tag="gt")
                    nc.gpsimd.dma_gather(
                        gt[:].rearrange("p (c f) -> p c f", f=128),
                        ag_out[q * wq : (q + 1) * wq, :],
                        it[:],
                        num_idxs=nidx,
                        num_idxs_reg=nidx,
                        elem_size=128,
                        elem_step=128,
                        single_packet=(nidx <= 1024),
                        queue_num=q,
                    )
                    gts.append(gt)

                mms = sched["mm_of_group"][g]
                m0 = sched["mm_base"][g]
                n_mm_g = len(mms)
                mt = metat.tile([128, n_mm_g], F32)
                nc.sync.dma_start(out=mt[:], in_=meta[:, m0 : m0 + n_mm_g])

                # wide one-hot builds over the group's matmul columns
                ohs = {}
                for w0 in range(0, n_mm_g, WOH):
                    wn = min(WOH, n_mm_g - w0)
                    oh = ohp.tile([128, wn * 128], BF16, name="oh", tag="oh")
                    nc.vector.tensor_tensor(
                        oh[:].rearrange("p (c f) -> p c f", f=128),
                        iota_sb[:, : wn * 128].rearrange("p (c f) -> p c f", f=128),
                        mt[:, w0 : w0 + wn].to_broadcast([128, wn, 128]),
                        mybir.AluOpType.is_equal,
                    )
                    ohs[w0] = oh

                # per-block accumulation
                k = 0
                for j in range(len(blocks)):
                    mm_j = [m for m in mms if m[0] == j]
                    if not mm_j:
                        continue
                    ps = agg_ps.tile([128, 128], F32, name="aggps", tag="aggps")
                    for i, (jj2, qq, c, _gc) in enumerate(mm_j):
                        col = k + i
                        w0 = (col // WOH) * WOH
                        off = col - w0
                        nc.tensor.matmul(
                            ps[:],
                            lhsT=gts[qq][:, c * 128 : (c + 1) * 128],
                            rhs=ohs[w0][:, off * 128 : (off + 1) * 128],
                            start=(i == 0),
                            stop=(i == len(mm_j) - 1),
                        )
                    k += len(mm_j)
                    b = blocks[j]
                    # out = psum * dis[dest] + bias
                    dd = ddp.tile([128, 128], F32, name="dd", tag="dd")
                    nc.sync.dma_start(
                        out=dd[:], in_=disd[:, b * 128 : (b + 1) * 128]
                    )
                    ob = outsb.tile([128, 128], F32, name="ob", tag="ob")
                    nc.vector.tensor_tensor(
                        ob[:], ps[:], dd[:], mybir.AluOpType.mult
                    )
                    nc.vector.tensor_scalar_add(ob[:], ob[:], bias(L))
                    nc.sync.dma_start(
                        out=h_out[
                            :, L * s_pad + b * 128 : L * s_pad + (b + 1) * 128
                        ],
                        in_=ob[:],
                    )
                    if L < 2:
                        r = rsb.tile([128, 128], BF16, name="r", tag="r")
                        nc.scalar.activation(
                            r[:], ob[:], mybir.ActivationFunctionType.Relu
                        )
                        ps2 = mm_ps.tile([128, 128], F32)
                        nc.tensor.matmul(
                            ps2[:],
                            lhsT=r[:],
                            rhs=w_bf[:, (L + 1) * 128 : (L + 2) * 128],
                            start=True,
                            stop=True,
                        )
                        hm = hmm.tile([128, 128], BF16)
                        nc.vector.tensor_scalar(
                            hm[:], ps2[:], dison_sb[:, b : b + 1], None,
                            mybir.AluOpType.mult,
                        )
                        nc.sync.dma_start(
                            out=ag_in[b * 128 : (b + 1) * 128, :], in_=hm[:]
                        )

    nc.compile()
    return nc


_BUILD_CACHE = {}


def _get_kernel(sched):
    key = (
        sched["nblk"], sched["s_pad"],
        tuple(tuple(c) for c in sched["caps"]),
        tuple(tuple(m) for g in sched["mm_of_group"] for m in g),
    )
    if key not in _BUILD_CACHE:
        _BUILD_CACHE[key] = build_kernel(sched)
    return _BUILD_CACHE[key]


# ----------------------------------------------------------------------------
# Entry point
# ----------------------------------------------------------------------------

def _run(x, edge_index, W0, b0, W1, b1, W2, b2, trace=False):
    n = int(np.asarray(x).shape[0])
    s_real = n // N_CORES
    in_maps, sched = _prep_inputs(
        x, edge_index, W0, b0, W1, b1, W2, b2, s_real
    )
    s_pad = sched["s_pad"]
    nc = _get_kernel(sched)
    res = bass_utils.run_bass_kernel_spmd(
        nc, in_maps, core_ids=list(range(N_CORES)), trace=trace
    )
    outs = []
    for L in range(3):
        h = np.concatenate(
            [
                res.results[r]["h_out"][:, L * s_pad : L * s_pad + s_real]
                for r in range(N_CORES)
            ],
            axis=1,
        ).T
        outs.append(h)
    full = np.stack(outs, axis=1).astype(np.float32)
    return full, res


def kernel(**inputs):
    trace = os.environ.get("TRN_KERNEL_TRACE", "") == "1"
    out, res = _run(
        np.asarray(inputs["x"]),
        np.asarray(inputs["edge_index"]),
        np.asarray(inputs["W0"]),
        np.asarray(inputs["b0"]),
        np.asarray(inputs["W1"]),
        np.asarray(inputs["b1"]),
        np.asarray(inputs["W2"]),
        np.asarray(inputs["b2"]),
        trace=trace,
    )
    if trace and res.exec_time_ns is not None:
        print(f"HW exec time: {res.exec_time_ns} ns")
        if res.instructions_and_trace:
            print(f"trace: {res.instructions_and_trace[1]}")
    return out
